# revision 1
# baseline (speedup 1.0000x reference)
"""Trainium2 Bass kernel for an 8-step complex DMD recurrence.

Math (matching the reference):
  Ag[0]=A[0], Ag[p]=A[8-p] (p>=1), all complex [M,M].
  uc window w_t (len 8) starts as the real inputs x_0..x_7; each step
    u2_t = sum_p Ag[p] @ w_t[p]   (complex, [B,M])
  then the window slides.  Output = Re([u2_1..u2_8]) as [B, 8, M].

Strategy (8 NeuronCores, tensor-parallel over output rows m):
  * core c owns m rows [128c, 128c+128); every core keeps the full batch
    B=256 as the matmul moving dimension.
  * x-only contributions are regrouped: v_t = sum_{pos<=8-t} Ag[pos] x_{pos+t-1}
    (x real => only 2 real matmuls per term, N=256, accumulated into one
    PSUM bank holding [v_re | v_im]).
  * recurrent terms u2_j (complex) use the [ur|ui] moving trick: one
    matmul with weights ArT -> bankA, one with AiT -> bankB, N=512.
    Combine: u2_re = v_re + bankA.l - bankB.r ; u2_im = v_im + bankA.r + bankB.l.
  * after each step an AllGather shares each core's [128,256] re/im slice
    so every core has the full u2_t for later steps.  The Tile scheduler
    overlaps the gathers with independent matmuls (later v-phase work and
    recurrent terms not involving the newest u2).
  * everything runs in float32r (full-rate fp32 matmul mode on trn2).
"""

import numpy as np

B, L, M = 256, 8, 1024
N_CORES = 8
NT = M // 128  # 8 contraction tiles of 128
P_STEPS = 8

_CACHE = {}


def _build_program():
    import concourse.bacc as bacc
    import concourse.mybir as mybir
    import concourse.tile as tile
    from concourse.bass import ts

    dt = mybir.dt
    fr = dt.float32r
    f32 = dt.float32

    nc = bacc.Bacc("TRN2", target_bir_lowering=False, debug=False,
                   num_devices=N_CORES)

    # Inputs (per core). Partition-major layouts prepared on the host:
    #   war/wai: [p, k, nt, m] -> [128, 8*8*128]   (AgT slices, this core's m)
    #   xw:      [p, q, nt, b] -> [128, 8*8*256]   (x transposed, full batch)
    war = nc.dram_tensor("war", [128, L * NT * 128], fr, kind="ExternalInput")
    wai = nc.dram_tensor("wai", [128, L * NT * 128], fr, kind="ExternalInput")
    xw = nc.dram_tensor("xw", [128, L * NT * 256], fr, kind="ExternalInput")
    out = nc.dram_tensor("out", [P_STEPS, 128, 256], f32, kind="ExternalOutput")

    # Collective buffers (one pair per gathered step).
    cc_in = [nc.dram_tensor(f"cc_in{t}", [256, 256], fr) for t in range(1, 8)]
    cc_out = [
        nc.dram_tensor(f"cc_out{t}", [256 * N_CORES, 256], fr, addr_space="Shared")
        for t in range(1, 8)
    ]

    rg = [list(range(N_CORES))]

    with tile.TileContext(nc) as tc:
        with (
            tc.tile_pool(name="a", bufs=1) as apool,
            tc.tile_pool(name="win", bufs=7) as wpool,
            tc.tile_pool(name="stg", bufs=4) as stpool,
            tc.tile_pool(name="vb", bufs=4, space="PSUM") as vbpool,
            tc.tile_pool(name="rb", bufs=4, space="PSUM") as rbpool,
        ):
            t_war = apool.tile([128, L * NT * 128], fr, tag="war")
            t_wai = apool.tile([128, L * NT * 128], fr, tag="wai")

            def wtile(which, pos, nt):
                t = t_war if which == 0 else t_wai
                return t[:, ts(pos * NT + nt, 128)]

            vbank = {}      # t -> psum tile [128,512] = [v_re | v_im]
            vstart = {}     # t -> whether bank already started
            slots = {}      # j -> sbuf tile [128, 8*512] ([ur|ui] per nt)

            def load_x(q):
                xt = wpool.tile([128, NT * 256], fr, tag="win")
                nc.sync.dma_start(xt[:], xw[:, ts(q, NT * 256)])
                return xt

            def v_terms(q, xt, t_list):
                for t in t_list:
                    pos = q - t + 1
                    if pos < 0 or pos > 8 - t:
                        continue
                    if t not in vbank:
                        vbank[t] = vbpool.tile([128, 512], f32, tag="vb", name=f"vb{t}")
                        vstart[t] = True
                    first = vstart[t]
                    vstart[t] = False
                    last = q == 7
                    for nt in range(NT):
                        rhs = xt[:, ts(nt, 256)]
                        nc.tensor.matmul(
                            vbank[t][:, 0:256], wtile(0, pos, nt), rhs,
                            start=(first and nt == 0),
                            stop=(last and nt == NT - 1),
                            skip_group_check=True,
                        )
                        # start only on the very first matmul into this bank
                        # (start marks the WHOLE 2KB zero-region pending; the
                        # right half's first write then overwrites-by-default)
                        nc.tensor.matmul(
                            vbank[t][:, 256:512], wtile(1, pos, nt), rhs,
                            start=False,
                            stop=(last and nt == NT - 1),
                            skip_group_check=True,
                        )

            # ---- pass A: v_1..v_4 (x positions 0..7) ----
            # A k-slices interleaved with x loads: position q only needs
            # Ag[pos<=q], so war[k=q] right before x_q keeps the first
            # matmuls from queueing behind the whole 8MB of A.
            xtiles = {}
            for q in range(0, 8):
                sl = ts(q, NT * 128)
                nc.sync.dma_start(t_war[:, sl], war[:, sl])
                nc.sync.dma_start(t_wai[:, sl], wai[:, sl])
                xt = load_x(q)
                xtiles[q] = xt
                v_terms(q, xt, [1, 2, 3, 4])

            def do_step(t):
                """Recurrent accumulation + combine + gather for step t."""
                if t == 1:
                    stg = stpool.tile([128, 512], f32, tag="stg")
                    nc.vector.tensor_copy(stg[:], vbank[1][:])
                else:
                    bankA = rbpool.tile([128, 512], f32, tag="rb")
                    bankB = rbpool.tile([128, 512], f32, tag="rb")
                    for j in range(1, t):
                        pos = 8 - t + j
                        for nt in range(NT):
                            rhs = slots[j][:, ts(nt, 512)]
                            nc.tensor.matmul(
                                bankA[:], wtile(0, pos, nt), rhs,
                                start=(j == 1 and nt == 0),
                                stop=(j == t - 1 and nt == NT - 1),
                                skip_group_check=True,
                            )
                            nc.tensor.matmul(
                                bankB[:], wtile(1, pos, nt), rhs,
                                start=(j == 1 and nt == 0),
                                stop=(j == t - 1 and nt == NT - 1),
                                skip_group_check=True,
                            )
                    stg = stpool.tile([128, 512], f32, tag="stg")
                    # DVE may read at most one PSUM operand per instruction.
                    # u2_re = bankA.l - bankB.r + v_re
                    # u2_im = bankA.r + bankB.l + v_im
                    nc.vector.tensor_copy(stg[:], bankA[:])
                    nc.vector.tensor_sub(stg[:, 0:256], stg[:, 0:256],
                                         bankB[:, 256:512])
                    nc.vector.tensor_add(stg[:, 256:512], stg[:, 256:512],
                                         bankB[:, 0:256])
                    nc.vector.tensor_add(stg[:], stg[:], vbank[t][:])
                del vbank[t]

                # this core's slice of Re(u2_t) -> output row t-1
                nc.sync.dma_start(out[t - 1], stg[:, 0:256])

                if t < 8:
                    ci, co = cc_in[t - 1], cc_out[t - 1]
                    nc.sync.dma_start(ci[0:128, :], stg[:, 0:256].bitcast(fr))
                    nc.sync.dma_start(ci[128:256, :], stg[:, 256:512].bitcast(fr))
                    nc.gpsimd.collective_compute(
                        "AllGather", mybir.AluOpType.bypass,
                        replica_groups=rg, ins=[ci[:]], outs=[co[:]],
                    )
                    slot = wpool.tile([128, NT * 512], fr, tag="win")
                    # split per gathered chunk: dependent matmuls on
                    # nt-slice c can start as soon as chunk c lands
                    for c in range(N_CORES):
                        src = co[c * 256:(c + 1) * 256, :].rearrange(
                            "(h p) b -> p h b", h=2, p=128)
                        dst = slot[:, c * 512:(c + 1) * 512].rearrange(
                            "p (h b) -> p h b", h=2, b=256)
                        nc.sync.dma_start(dst, src)
                    slots[t] = slot

            do_step(1)

            # ---- pass B: v_5..v_8 (x positions 4..7, tiles kept from
            # pass A -- no reload; pool occupancy stays <= 7 slots) ----
            for q in range(4, 8):
                v_terms(q, xtiles[q], [5, 6, 7, 8])

            for t in range(2, 9):
                do_step(t)

    nc.compile()
    return nc


def _get_runner():
    if "runner" in _CACHE:
        return _CACHE["runner"]

    import jax
    from jax.sharding import Mesh, PartitionSpec
    from jax.experimental.shard_map import shard_map
    import concourse.mybir as mybir
    from concourse import bass2jax

    nc = _build_program()
    bass2jax.install_neuronx_cc_hook()
    partition_name = nc.partition_id_tensor.name if nc.partition_id_tensor else None
    in_names, out_names, out_avals, zero_outs = [], [], [], []
    for alloc in nc.m.functions[0].allocations:
        if not isinstance(alloc, mybir.MemoryLocationSet):
            continue
        name = alloc.memorylocations[0].name
        if alloc.kind == "ExternalInput":
            if name != partition_name:
                in_names.append(name)
        elif alloc.kind == "ExternalOutput":
            out_names.append(name)
            shape = tuple(alloc.tensor_shape)
            dtype = mybir.dt.np(alloc.dtype)
            out_avals.append(jax.core.ShapedArray(shape, dtype))
            zero_outs.append(np.zeros(shape, dtype))
    n_params = len(in_names)
    n_outs = len(out_avals)
    all_in = in_names + out_names + ([partition_name] if partition_name else [])
    donate = tuple(range(n_params, n_params + n_outs))

    def _body(*args):
        operands = list(args)
        if partition_name is not None:
            operands.append(bass2jax.partition_id_tensor())
        return tuple(
            bass2jax._bass_exec_p.bind(
                *operands,
                out_avals=tuple(out_avals),
                in_names=tuple(all_in),
                out_names=tuple(out_names),
                lowering_input_output_aliases=(),
                sim_require_finite=True,
                sim_require_nnan=True,
                nc=nc,
            )
        )

    devices = jax.devices()[:N_CORES]
    mesh = Mesh(np.asarray(devices), ("core",))
    sharded = jax.jit(
        shard_map(
            _body, mesh=mesh,
            in_specs=(PartitionSpec("core"),) * (n_params + n_outs),
            out_specs=(PartitionSpec("core"),) * n_outs,
            check_rep=False,
        ),
        donate_argnums=donate,
        keep_unused=True,
    )
    runner = {
        "sharded": sharded,
        "in_names": in_names,
        "out_names": out_names,
        "out_avals": out_avals,
        "zero_outs": zero_outs,
        "mesh": mesh,
    }
    _CACHE["runner"] = runner
    return runner


def prepare_inputs(x, A_real, A_imag):
    """Host-side reorder/transpose into the kernel's DMA-friendly layouts."""
    x = np.asarray(x, dtype=np.float32)
    A_real = np.asarray(A_real, dtype=np.float32)
    A_imag = np.asarray(A_imag, dtype=np.float32)
    idx = np.concatenate([[0], np.arange(L - 1, 0, -1)]).astype(np.int64)
    Agr = A_real[idx]  # [k, m, n]
    Agi = A_imag[idx]
    # AgT: [k, n, m]; per-core slice of m; partition-major [p, k, nt, m]
    AgrT = np.ascontiguousarray(Agr.transpose(0, 2, 1))
    AgiT = np.ascontiguousarray(Agi.transpose(0, 2, 1))
    wars, wais = [], []
    for c in range(N_CORES):
        sl = AgrT[:, :, c * 128:(c + 1) * 128]  # [k, n, 128]
        w = sl.reshape(L, NT, 128, 128).transpose(2, 0, 1, 3).reshape(128, -1)
        wars.append(np.ascontiguousarray(w))
        sl = AgiT[:, :, c * 128:(c + 1) * 128]
        w = sl.reshape(L, NT, 128, 128).transpose(2, 0, 1, 3).reshape(128, -1)
        wais.append(np.ascontiguousarray(w))
    # x: [b, q, m] -> [p, q, nt, b]
    xt = x.transpose(1, 2, 0).reshape(L, NT, 128, B)
    xw = np.ascontiguousarray(xt.transpose(2, 0, 1, 3).reshape(128, -1))
    return wars, wais, xw


def kernel(x, A_real, A_imag, predict_length):
    P = int(predict_length)
    if P != P_STEPS:  # pragma: no cover - reference always uses 8
        return _numpy_fallback(x, A_real, A_imag, P)

    import jax

    runner = _get_runner()
    wars, wais, xw = prepare_inputs(x, A_real, A_imag)
    in_maps = [{"war": wars[c], "wai": wais[c], "xw": xw} for c in range(N_CORES)]
    concat_in = [
        np.concatenate([m[n] for m in in_maps], axis=0) for n in runner["in_names"]
    ]
    czeros = [
        np.zeros((N_CORES * z.shape[0], *z.shape[1:]), z.dtype)
        for z in runner["zero_outs"]
    ]
    out_arrs = runner["sharded"](*concat_in, *czeros)
    jax.block_until_ready(out_arrs)
    o = np.asarray(out_arrs[0]).reshape(N_CORES, P_STEPS, 128, 256)
    # [c, t, p, b] -> [b, t, c*128+p]
    full = o.transpose(3, 1, 0, 2).reshape(B, P_STEPS, M)
    return np.ascontiguousarray(full.astype(np.float32))


def _numpy_fallback(x, A_real, A_imag, P):
    A = (np.asarray(A_real) + 1j * np.asarray(A_imag)).astype(np.complex64)
    idx = np.concatenate([[0], np.arange(L - 1, 0, -1)]).astype(np.int64)
    Ag = A[idx]
    uc = np.asarray(x).astype(np.complex64)
    for _ in range(P):
        u2 = np.einsum("kmn,bkn->bm", Ag, uc)
        uc = np.concatenate([uc[:, 1:], u2[:, None]], axis=1)
    return np.real(uc).astype(np.float32)

